# revision 11
# baseline (speedup 1.0000x reference)
"""TRN2 Bass kernel for nn_BlockPermProduct.

The reference applies 9 probabilistic block-permutation mixing steps to each
row of x [65536, 1024]. Every step is linear in x, so the whole transform is
``out = x @ M^T`` for a 1024x1024 matrix M depending only on the (9, 3)
logits; M is built on the host in float64 by pushing the identity through the
reference transform.

Structure exploited:

1. Exact block sparsity. Under the feature grouping g = b0 + 2*b1 + 4*b9
   (bits of the feature index), M has exact zero blocks: outputs in the first
   half (b9=0) never depend on inputs with (b9=1 & b0=1), and outputs in the
   second half never depend on inputs with (b9=0 & b0=0). Each 128-feature
   output block contracts only 6 of the 8 input groups: 48 K-block matmuls
   per 512 rows instead of the dense 64.

2. Host-side transposes. x is transposed on the host (per core) to
   x_t [1024, 8192], so feature-major tiles DMA straight into SBUF and the
   PE does only matmuls — no on-chip transposes. The output is produced
   transposed (out_t [1024, 8192]) and un-transposed on the host.

3. Weight-load amortization. The matmul moving width is ISA-capped at 512,
   and each weight swap costs ~108 idle PE cycles, so steps cover 1024 rows
   and each (in-group, out-block) weight block is used by two back-to-back
   512-wide matmuls (row halves) — if the PE only flushes on weight swap,
   the second matmul of each pair chains without the bubble.

Everything runs in bf16 (rel err ~4e-3 vs the 2e-2 gate) with fp32 PSUM
accumulation; bf16 I/O halves both DMA directions. PSUM->SBUF casts
alternate between the DVE and ACT engines. DMA transfers are 128-256 KiB
and the first x chunk / M tiles are interleaved so compute starts ~8 us in.

Sharding: pure data parallel over the batch dim across 8 cores (SPMD, no
communication); M is replicated.
"""

import numpy as np
from contextlib import ExitStack

import ml_dtypes

import concourse.bass as bass
import concourse.bacc as bacc
import concourse.mybir as mybir
import concourse.tile as tile
from concourse.bass_utils import run_bass_kernel_spmd

BATCH = 65536
SIZE = 1024
N_CORES = 8
ROWS_PER_CORE = BATCH // N_CORES  # 8192
P = 128
RW = 512  # ISA max matmul moving width
NMAX = 1024  # rows per step (two RW halves share each weight load)
SCHED = [512] + [1024] * 7 + [512]
assert sum(SCHED) == ROWS_PER_CORE

F32 = mybir.dt.float32
BF16 = mybir.dt.bfloat16
NP_BF16 = ml_dtypes.bfloat16

# Input-group lists per b9-half of the output (verified against M at runtime;
# dense fallback if the zero pattern does not hold).
KO_HALF0 = [0, 1, 2, 3, 4, 6]  # skip in-groups 5,7 (b9=1 & b0=1)
KO_HALF1 = [1, 3, 4, 5, 6, 7]  # skip in-groups 0,2 (b9=0 & b0=0)

TRACE = False
TRACE_KWARGS = {}
LAST_RESULTS = None

_NC_CACHE = {}


def _transform64(y, logits):
    """Float64 port of the reference transform, applied to rows of y."""
    m = 10
    sizes = [SIZE >> i for i in range(m - 1)][::-1]  # [4, 8, ..., 1024]
    out = y
    for i in range(m - 2, -1, -1):
        n = sizes[i]
        p = 1.0 / (1.0 + np.exp(-logits[i].astype(np.float64)))
        z = out.reshape(-1, n)
        sep = z.reshape(-1, n // 2, 2).transpose(0, 2, 1).reshape(-1, n)
        z = (1 - p[0]) * z + p[0] * sep
        h = n // 2
        first = (1 - p[1]) * z[:, :h] + p[1] * z[:, h - 1::-1]
        second = (1 - p[2]) * z[:, h:] + p[2] * z[:, : h - 1 : -1]
        out = np.concatenate([first, second], axis=1).reshape(out.shape)
    return out


def _build_m(logits):
    """M [1024, 1024] float64: out_row = M @ x_row."""
    eye = np.eye(SIZE, dtype=np.float64)
    mt = _transform64(eye, logits)  # row j = M column j
    return mt.T


def _feat(g, f):
    """Global feature index of element f (0..127) of group g (0..7)."""
    return 512 * (g >> 2) + 4 * f + (g & 3)


_GROUP_FEATS = [np.array([_feat(g, f) for f in range(P)]) for g in range(8)]


def _check_sparse(m):
    """True iff the 48-block zero pattern holds for this M."""
    for o in range(8):
        rows = _GROUP_FEATS[o]
        banned = [5, 7] if o < 4 else [0, 2]
        for i in banned:
            cols = _GROUP_FEATS[i]
            if np.abs(m[np.ix_(rows, cols)]).max() > 1e-12:
                return False
    return True


def _build_mtg(m):
    """Grouped M operand [1024, 1024] bf16.

    mtg[i*128 + f, o*128 + c] = M[_feat(o, c), _feat(i, f)]: row blocks are
    input groups (the matmul contraction dim), column blocks are output
    groups (the matmul stationary free dim).
    """
    mtg = np.zeros((SIZE, SIZE), dtype=np.float64)
    for i in range(8):
        cols = _GROUP_FEATS[i]
        for o in range(8):
            rows = _GROUP_FEATS[o]
            mtg[i * P : (i + 1) * P, o * P : (o + 1) * P] = m[
                np.ix_(rows, cols)
            ].T
    return np.ascontiguousarray(mtg.astype(NP_BF16))


def _build_bass(sparse):
    ko_half = [KO_HALF0, KO_HALF1] if sparse else [list(range(8))] * 2
    nc = bacc.Bacc("TRN2", target_bir_lowering=False, debug=False)
    # x_t / out_t are the per-core transposes: [feature, row].
    xt = nc.dram_tensor("xt", [SIZE, ROWS_PER_CORE], BF16, kind="ExternalInput").ap()
    mtg = nc.dram_tensor("mtg", [SIZE, SIZE], BF16, kind="ExternalInput").ap()
    out = nc.dram_tensor(
        "out_t", [SIZE, ROWS_PER_CORE], BF16, kind="ExternalOutput"
    ).ap()

    with tile.TileContext(nc) as tc, ExitStack() as ctx:
        const = ctx.enter_context(tc.tile_pool(name="const", bufs=1))
        xpool = ctx.enter_context(tc.tile_pool(name="xin", bufs=3))

        def x_dmas(t, r0, n, chunk):
            # Emit (dst, src) AP pairs: group-pair x row-chunk transfers.
            tv = t[:].rearrange("p (g r) -> p g r", g=8)
            rsplit = max(1, n // chunk)
            rc = n // rsplit
            pairs = []
            for gs in range(4):
                h, q0 = divmod(2 * gs, 4)
                src = xt[512 * h : 512 * (h + 1), r0 : r0 + n].rearrange(
                    "(f q) r -> f q r", q=4
                )
                for rr in range(rsplit):
                    pairs.append(
                        (
                            tv[:, 2 * gs : 2 * gs + 2, rr * rc : (rr + 1) * rc],
                            src[:, q0 : q0 + 2, rr * rc : (rr + 1) * rc],
                        )
                    )
            return pairs

        # First x chunk and M tiles interleaved across queues so the first
        # matmul (o=0: needs x group 0 + mtg[i=0, o<4]) starts ~8 us in.
        xin0 = xpool.tile([P, 8 * NMAX], BF16, tag="xin")
        x0 = x_dmas(xin0, 0, SCHED[0], 128)
        mts = []
        for i in range(8):
            t = const.tile([P, SIZE], BF16, tag=f"mt{i}")
            mts.append(t)
        mt_dmas = []
        for hh in range(2):
            for i in range(8):
                mt_dmas.append(
                    (
                        mts[i][:, hh * 512 : (hh + 1) * 512],
                        mtg[i * P : (i + 1) * P, hh * 512 : (hh + 1) * 512],
                    )
                )
        for k in range(max(len(x0), len(mt_dmas))):
            if k < len(x0):
                nc.sync.dma_start(*x0[k])
            if k < len(mt_dmas):
                nc.sync.dma_start(*mt_dmas[k])

        opool = ctx.enter_context(tc.tile_pool(name="osb", bufs=2))
        # po = one out-block [128, NMAX] fp32 (2 banks); 2 pools x 2 bufs = 8.
        psa = ctx.enter_context(tc.tile_pool(name="psa", bufs=2, space="PSUM"))
        psb = ctx.enter_context(tc.tile_pool(name="psb", bufs=2, space="PSUM"))

        r0 = 0
        for step, n in enumerate(SCHED):
            if step == 0:
                xin = xin0
            else:
                xin = xpool.tile([P, 8 * NMAX], BF16, tag="xin")
                for d in x_dmas(xin, r0, n, 512):
                    nc.sync.dma_start(*d)
            osb = opool.tile([P, 8 * NMAX], BF16, tag="osb")
            ov = osb[:].rearrange("p (g r) -> p g r", g=8)
            nrr = n // RW

            for o in range(8):
                ko = ko_half[o >> 2]
                po = (psa if o % 2 == 0 else psb).tile([P, NMAX], F32, tag="po")
                for idx, i in enumerate(ko):
                    # Both row-halves back-to-back under one weight load.
                    for rr in range(nrr):
                        nc.tensor.matmul(
                            po[:, rr * RW : (rr + 1) * RW],
                            mts[i][:, o * P : (o + 1) * P],
                            xin[:, i * NMAX + rr * RW : i * NMAX + rr * RW + RW],
                            start=(idx == 0),
                            stop=(idx == len(ko) - 1),
                        )
                if o % 2 == 0:
                    nc.vector.tensor_copy(ov[:, o, :n], po[:, :n])
                else:
                    nc.scalar.copy(ov[:, o, :n], po[:, :n])

            # Stores: 256 KiB chunks; the last step drains in 128 KiB chunks.
            rsplit = max(1, n // (256 if step == len(SCHED) - 1 else 512))
            rc = n // rsplit
            for gs in range(4):
                h, q0 = divmod(2 * gs, 4)
                dst = out[512 * h : 512 * (h + 1), r0 : r0 + n].rearrange(
                    "(c q) r -> c q r", q=4
                )
                for rr in range(rsplit):
                    nc.sync.dma_start(
                        dst[:, q0 : q0 + 2, rr * rc : (rr + 1) * rc],
                        ov[:, 2 * gs : 2 * gs + 2, rr * rc : (rr + 1) * rc],
                    )
            r0 += n

    _dedup_ldweights(nc)
    nc.compile()
    return nc


def _dedup_ldweights(nc):
    """Remove back-to-back InstLdweights with identical weight APs.

    The tile pass emits one Ldweights per matmul even when consecutive
    matmuls reuse the same stationary operand (our row-half pairs). The PE
    flushes its pipeline on every weight swap (~108 cycles), so dropping the
    redundant reload lets the second matmul of each pair chain bubble-free.
    Only instructions with no semaphore waits/updates are removed.
    """
    removed = 0
    for fn in nc.m.functions:
        for blk in fn.blocks:
            insts = blk.instructions
            prev_key = None
            drop = []
            for idx in range(len(insts)):
                inst = insts[idx]
                if type(inst).__name__ != "InstLdweights":
                    continue
                key = repr(inst.ins[0])
                si = inst.sync_info
                clean = si is None or (not si.on_wait and not si.on_update)
                if key == prev_key and clean:
                    drop.append(idx)
                else:
                    prev_key = key
            for idx in reversed(drop):
                del insts[idx]
            removed += len(drop)
    return removed


def _get_nc(sparse):
    key = bool(sparse)
    if key not in _NC_CACHE:
        _NC_CACHE[key] = _build_bass(key)
    return _NC_CACHE[key]


def kernel(x, logits):
    x = np.asarray(x)
    logits = np.asarray(logits)
    assert x.shape == (BATCH, SIZE)

    m = _build_m(logits)
    sparse = _check_sparse(m)
    mtg = _build_mtg(m)
    nc = _get_nc(sparse)

    xb = x.astype(NP_BF16)
    in_maps = [
        {
            "xt": np.ascontiguousarray(
                xb[i * ROWS_PER_CORE : (i + 1) * ROWS_PER_CORE].T
            ),
            "mtg": mtg,
        }
        for i in range(N_CORES)
    ]
    kwargs = dict(TRACE_KWARGS)
    if TRACE:
        kwargs.setdefault("trace", True)
        kwargs.setdefault("trace_cores", [0])
    res = run_bass_kernel_spmd(nc, in_maps, core_ids=list(range(N_CORES)), **kwargs)
    global LAST_RESULTS
    LAST_RESULTS = res
    outs = [np.asarray(res.results[i]["out_t"]).T for i in range(N_CORES)]
    return np.ascontiguousarray(np.concatenate(outs, axis=0)).astype(np.float32)


# revision 14
# speedup vs baseline: 1.1646x; 1.1646x over previous
"""TRN2 Bass kernel for nn_BlockPermProduct — v3 fallback (measured 228411 ns).

out = x @ M^T via 48 block-sparse bf16 matmuls per 512 rows; x pre-transposed
on the host so the PE does matmuls only. See kernel.py for the full notes.
"""

import numpy as np
from contextlib import ExitStack

import ml_dtypes

import concourse.bass as bass
import concourse.bacc as bacc
import concourse.mybir as mybir
import concourse.tile as tile
from concourse.bass_utils import run_bass_kernel_spmd

BATCH = 65536
SIZE = 1024
N_CORES = 8
ROWS_PER_CORE = BATCH // N_CORES  # 8192
P = 128
RW = 512
N_STEPS = ROWS_PER_CORE // RW  # 16

F32 = mybir.dt.float32
BF16 = mybir.dt.bfloat16
NP_BF16 = ml_dtypes.bfloat16

KO_HALF0 = [0, 1, 2, 3, 4, 6]
KO_HALF1 = [1, 3, 4, 5, 6, 7]

TRACE = False
TRACE_KWARGS = {}
LAST_RESULTS = None

_NC_CACHE = {}


def _transform64(y, logits):
    m = 10
    sizes = [SIZE >> i for i in range(m - 1)][::-1]
    out = y
    for i in range(m - 2, -1, -1):
        n = sizes[i]
        p = 1.0 / (1.0 + np.exp(-logits[i].astype(np.float64)))
        z = out.reshape(-1, n)
        sep = z.reshape(-1, n // 2, 2).transpose(0, 2, 1).reshape(-1, n)
        z = (1 - p[0]) * z + p[0] * sep
        h = n // 2
        first = (1 - p[1]) * z[:, :h] + p[1] * z[:, h - 1::-1]
        second = (1 - p[2]) * z[:, h:] + p[2] * z[:, : h - 1 : -1]
        out = np.concatenate([first, second], axis=1).reshape(out.shape)
    return out


def _build_m(logits):
    eye = np.eye(SIZE, dtype=np.float64)
    mt = _transform64(eye, logits)
    return mt.T


def _feat(g, f):
    return 512 * (g >> 2) + 4 * f + (g & 3)


_GROUP_FEATS = [np.array([_feat(g, f) for f in range(P)]) for g in range(8)]


def _check_sparse(m):
    for o in range(8):
        rows = _GROUP_FEATS[o]
        banned = [5, 7] if o < 4 else [0, 2]
        for i in banned:
            cols = _GROUP_FEATS[i]
            if np.abs(m[np.ix_(rows, cols)]).max() > 1e-12:
                return False
    return True


def _build_mtg(m):
    mtg = np.zeros((SIZE, SIZE), dtype=np.float64)
    for i in range(8):
        cols = _GROUP_FEATS[i]
        for o in range(8):
            rows = _GROUP_FEATS[o]
            mtg[i * P : (i + 1) * P, o * P : (o + 1) * P] = m[
                np.ix_(rows, cols)
            ].T
    return np.ascontiguousarray(mtg.astype(NP_BF16))


def _build_bass(sparse):
    ko_half = [KO_HALF0, KO_HALF1] if sparse else [list(range(8))] * 2
    nc = bacc.Bacc("TRN2", target_bir_lowering=False, debug=False)
    xt = nc.dram_tensor("xt", [SIZE, ROWS_PER_CORE], BF16, kind="ExternalInput").ap()
    mtg = nc.dram_tensor("mtg", [SIZE, SIZE], BF16, kind="ExternalInput").ap()
    out = nc.dram_tensor(
        "out_t", [SIZE, ROWS_PER_CORE], BF16, kind="ExternalOutput"
    ).ap()

    with tile.TileContext(nc) as tc, ExitStack() as ctx:
        const = ctx.enter_context(tc.tile_pool(name="const", bufs=1))
        xpool = ctx.enter_context(tc.tile_pool(name="xin", bufs=4))

        def x_dmas(t, r0, chunk):
            # (dst, src) AP pairs: group-pair x row-chunk transfers.
            pairs = []
            rsplit = RW // chunk
            for gs in range(4):
                h, q0 = divmod(2 * gs, 4)
                src = xt[512 * h : 512 * (h + 1), r0 : r0 + RW].rearrange(
                    "(f q) r -> f q r", q=4
                )
                dstv = t[:, 2 * gs * RW : 2 * (gs + 1) * RW].rearrange(
                    "p (q r) -> p q r", q=2
                )
                for rr in range(rsplit):
                    pairs.append(
                        (
                            dstv[:, :, rr * chunk : (rr + 1) * chunk],
                            src[:, q0 : q0 + 2, rr * chunk : (rr + 1) * chunk],
                        )
                    )
            return pairs

        def load_x(r0, chunk=512):
            t = xpool.tile([P, 8 * RW], BF16, tag="xin")
            for d in x_dmas(t, r0, chunk):
                nc.sync.dma_start(*d)
            return t

        # First x block in 128 KiB chunks, interleaved with the M tiles
        # (also 128 KiB halves) so the first matmuls start ~8 us in.
        xin0 = xpool.tile([P, 8 * RW], BF16, tag="xin")
        x0 = x_dmas(xin0, 0, 128)
        mts = []
        mt_dmas = []
        for i in range(8):
            t = const.tile([P, SIZE], BF16, tag=f"mt{i}")
            mts.append(t)
        for hh in range(2):
            for i in range(8):
                mt_dmas.append(
                    (
                        mts[i][:, hh * 512 : (hh + 1) * 512],
                        mtg[i * P : (i + 1) * P, hh * 512 : (hh + 1) * 512],
                    )
                )
        for k in range(max(len(x0), len(mt_dmas))):
            if k < len(x0):
                nc.sync.dma_start(*x0[k])
            if k < len(mt_dmas):
                nc.sync.dma_start(*mt_dmas[k])

        opool = ctx.enter_context(tc.tile_pool(name="osb", bufs=3))
        pso = ctx.enter_context(tc.tile_pool(name="pso", bufs=1, space="PSUM"))

        for step in range(N_STEPS):
            r0 = step * RW
            xin = xin0 if step == 0 else load_x(r0)
            osb = opool.tile([P, 8 * RW], BF16, tag="osb")
            ov = osb[:].rearrange("p (g r) -> p g r", g=8)

            for h in range(2):
                ko = ko_half[h]
                po = pso.tile([P, 4 * RW], F32, tag=f"po{h}")
                for q in range(4):
                    o = 4 * h + q
                    for idx, i in enumerate(ko):
                        nc.tensor.matmul(
                            po[:, q * RW : (q + 1) * RW],
                            mts[i][:, o * P : (o + 1) * P],
                            xin[:, i * RW : (i + 1) * RW],
                            start=(idx == 0),
                            stop=(idx == len(ko) - 1),
                        )
                # PSUM->SBUF casts split across DVE (h=0) and ACT (h=1).
                if h == 0:
                    nc.vector.tensor_copy(
                        osb[:, h * 4 * RW : (h + 1) * 4 * RW], po[:]
                    )
                else:
                    nc.scalar.copy(osb[:, h * 4 * RW : (h + 1) * 4 * RW], po[:])

            # Stores in 256 KiB chunks; the final step drains in 128 KiB.
            rsplit = 2 if step == N_STEPS - 1 else 1
            rc = RW // rsplit
            for gs in range(4):
                h, q0 = divmod(2 * gs, 4)
                dst = out[512 * h : 512 * (h + 1), r0 : r0 + RW].rearrange(
                    "(c q) r -> c q r", q=4
                )
                for rr in range(rsplit):
                    nc.sync.dma_start(
                        dst[:, q0 : q0 + 2, rr * rc : (rr + 1) * rc],
                        ov[:, 2 * gs : 2 * gs + 2, rr * rc : (rr + 1) * rc],
                    )

    nc.compile()
    return nc


def _get_nc(sparse):
    key = bool(sparse)
    if key not in _NC_CACHE:
        _NC_CACHE[key] = _build_bass(key)
    return _NC_CACHE[key]


def kernel(x, logits):
    x = np.asarray(x)
    logits = np.asarray(logits)
    assert x.shape == (BATCH, SIZE)

    m = _build_m(logits)
    sparse = _check_sparse(m)
    mtg = _build_mtg(m)
    nc = _get_nc(sparse)

    xb = x.astype(NP_BF16)
    in_maps = [
        {
            "xt": np.ascontiguousarray(
                xb[i * ROWS_PER_CORE : (i + 1) * ROWS_PER_CORE].T
            ),
            "mtg": mtg,
        }
        for i in range(N_CORES)
    ]
    kwargs = dict(TRACE_KWARGS)
    if TRACE:
        kwargs.setdefault("trace", True)
        kwargs.setdefault("trace_cores", [0])
    res = run_bass_kernel_spmd(nc, in_maps, core_ids=list(range(N_CORES)), **kwargs)
    global LAST_RESULTS
    LAST_RESULTS = res
    outs = [np.asarray(res.results[i]["out_t"]).T for i in range(N_CORES)]
    return np.ascontiguousarray(np.concatenate(outs, axis=0)).astype(np.float32)
